# revision 1
# baseline (speedup 1.0000x reference)
"""Trainium2 Bass kernel for: relu(1 - beta + x @ W^T).

Shapes (hardcoded): x [4096, 4096] f32, weights [4096, 4096] f32, beta [1] f32.
Output: [4096, 4096] f32.

Strategy: 8 cores as a 4 (batch) x 2 (output) grid. Host pre-transposes x/W to
fp16 so the contraction dim (IN) lands on SBUF partitions with contiguous DMA;
matmuls run fp16 x fp16 -> fp32 PSUM (~2.5e-4 rel err), the ReLU + (1-beta)
bias epilogue reads PSUM on ScalarE/VectorE. Raw Bacc (no Tile) with
hand-rolled semaphores and a minimal exit sequence.

Engine roles:
  sync   — all w-tile loads AND all output stores (HWDGE)
  gpsimd — x loads (SWDGE), final completion waits + semaphore teardown
  tensor — 1024 matmuls
  scalar — ReLU+bias epilogue for even m + two startup x chunks
  vector — bias compute + ReLU+bias epilogue for odd m

No explicit barrier at the end: each engine's (Bacc-emitted) cleanup runs as
soon as that engine's work is done, overlapping the final DMA drain. gpsimd
gates teardown on the store-completion semaphores alone.

Parameterized sizes so a miniature version can be validated in CoreSim.
"""
import numpy as np

import concourse.bass as bass
import concourse.mybir as mybir
from concourse import bacc

F32 = mybir.dt.float32
F16 = mybir.dt.float16
N_WARMUP_MM = 36   # dummy PE matmuls at stream start to warm the HAM clock
                   # (~3.9 us of N=128 matmuls bridges the startup DMA wait,
                   # so the HAM is at full clock when real data lands)


def build_raw(IN=4096, MB=1024, NO=2048, W_BUFS=12, safe_exit=False):
    KT = IN // 128          # contraction tiles
    NT = NO // 512          # output-col passes
    MT = MB // 128          # batch-row tiles (psum banks used)
    assert MT <= 8 and MT % 2 == 0 and NT >= 2
    NW = NT * KT            # total w tiles

    nc = bacc.Bacc("TRN2", target_bir_lowering=False, debug=False)
    xT = nc.dram_tensor("xT", [IN, MB], F16, kind="ExternalInput").ap()
    wT = nc.dram_tensor("wT", [IN, NO], F16, kind="ExternalInput").ap()
    beta = nc.dram_tensor("beta", [128, 1], F32, kind="ExternalInput").ap()
    out = nc.dram_tensor("out", [MB, NO], F16, kind="ExternalOutput").ap()

    x_sb = nc.alloc_sbuf_tensor("x_sb", [128, KT, MB], F16).ap()
    w_sb = nc.alloc_sbuf_tensor("w_sb", [128, W_BUFS, 512], F16).ap()
    o_sb = nc.alloc_sbuf_tensor("o_sb", [128, 2, MT, 512], F16).ap()
    beta_sb = nc.alloc_sbuf_tensor("beta_sb", [128, 1], F32).ap()
    bias_sb = nc.alloc_sbuf_tensor("bias_sb", [128, 1], F32).ap()
    ps = nc.alloc_psum_tensor("ps", [128, MT, 512], F32).ap()

    # ---- semaphores ----
    first_sem = None

    def sem(name):
        nonlocal first_sem
        s = nc.alloc_semaphore(name)
        if first_sem is None:
            first_sem = s
        return s

    s_x = [sem(f"s_x{k}") for k in range(KT)]        # x tile arrivals (gpsimd SWDGE)
    s_xs = [sem(f"s_xs{k}") for k in range(4)]       # scalar-issued startup x halves
    s_w = [sem(f"s_w{s}") for s in range(W_BUFS)]    # w slot arrivals (sync HWDGE)
    s_wh = [sem("s_wh0"), sem("s_wh1")]              # 2nd chunk of w tiles 0/1
    s_wu = sem("s_wu")                               # w tiles consumed (PE, +1)
    s_mm = sem("s_mm")                               # (j,m) accum groups done (+1)
    s_eps = sem("s_eps")                             # scalar epilogue ops (+1)
    s_epv = sem("s_epv")                             # vector epilogue ops (+1)
    s_o = [sem("s_o0"), sem("s_o1")]                 # store completions per o-slot
    s_b = sem("s_b")                                 # beta arrival
    s_bias = sem("s_bias")                           # bias computed
    s_fin = sem("s_fin")                             # scalar+vector final relay
    last_sem = s_fin
    sem_range = range(first_sem.num, last_sem.num + 1)
    # store sems live outside the main range: cleared in a late second
    # teardown so the main semaphore reset is off the store-drain path
    s_oS = sem("s_oS")      # sync-issued last-pass stores (HWDGE)
    s_oG = sem("s_oG")      # gpsimd-issued last-pass stores (SWDGE)
    s_sd = sem("s_sd")      # sync drained relay (engine inc)
    late_range = range(s_oS.num, s_sd.num + 1)

    # Early x tiles are split into two halves loaded by different engines
    # (gpsimd: cols [0, MB/2), scalar: cols [MB/2, MB)), each signalled by a
    # single DMA so chunk-completion order across queues can't invert.
    N_SPLIT_X = 4   # k-tiles with the 2-engine half split

    # number of w DMA chunks for tile index i (j=0 early tiles split)
    def w_chunks(i):
        return 2 if i < 2 else 1

    # cumulative inc target for w slot when consuming tile index i
    # one s_w inc per tile: the 2nd chunks of the split tiles 0/1 signal
    # their own s_wh sems so chunk-completion order can't be confused
    w_slot_target = [0] * W_BUFS
    w_targets = []
    for i in range(NW):
        sl = i % W_BUFS
        w_slot_target[sl] += 16
        w_targets.append(w_slot_target[sl])

    # store accounting: only mid-pass stores (gpsimd, 2 DMAs each) carry
    # semaphores. Last-pass stores are sem-free: data landing before NEFF
    # end is guaranteed by Bacc's exit-sequence per-engine DRAIN, which
    # waits out the issuing engine's DGE queues. This keeps the semaphore
    # teardown off the store-drain critical path.
    o_slot_cum = [0, 0]
    o_targets = []                        # cumulative per slot AFTER each pass
    for j in range(NT - 1):
        o_slot_cum[j % 2] += 32
        o_targets.append(o_slot_cum[j % 2])

    # epilogue inc target for (j, m): scalar does even m, vector odd
    def ep_wait(j, m):
        if m % 2 == 0:
            return s_eps, (MT // 2) * j + m // 2 + 1
        return s_epv, (MT // 2) * j + (m - 1) // 2 + 1

    def emit_store_pass(eng, j):
        """Both 4-m halves of pass j as two DMAs (used for j < NT-1)."""
        eng.wait_ge(s_eps, (MT // 2) * (j + 1))
        eng.wait_ge(s_epv, (MT // 2) * (j + 1))
        half = MT // 2
        for h in range(2):
            eng.dma_start(
                out[h * half * 128:(h + 1) * half * 128,
                    j * 512:(j + 1) * 512].rearrange("(m p) c -> p m c", p=128),
                o_sb[:, j % 2, h * half:(h + 1) * half, :],
            ).then_inc(s_o[j % 2], 16)

    with nc.Block() as block:

        @block.sync
        def _(sync: bass.BassEngine):
            i = 0
            for j in range(NT):
                for kt in range(KT):
                    sl = i % W_BUFS
                    if i >= W_BUFS:
                        sync.wait_ge(s_wu, i - W_BUFS + 1)
                    nch = w_chunks(i)
                    cw = 512 // nch
                    for ci in range(nch):
                        sync.dma_start(
                            w_sb[:, sl, ci * cw:(ci + 1) * cw],
                            wT[kt * 128:(kt + 1) * 128,
                               j * 512 + ci * cw:j * 512 + (ci + 1) * cw],
                        ).then_inc(s_wh[i] if (i < 2 and ci == 1)
                                   else s_w[sl], 16)
                    i += 1
                    if i == 3:
                        # beta load off the critical first-w path
                        sync.dma_start(beta_sb[:], beta[:]).then_inc(s_b, 16)
            # last pass: stores spread over 3 DMA-capable queues
            # (sync m0/m1/m4, gpsimd m3/m5/m7, scalar m2/m6 inline)
            j = NT - 1
            for m in (0, 1, MT - 4):
                wsem, wval = ep_wait(j, m)
                sync.wait_ge(wsem, wval)
                sync.dma_start(
                    out[m * 128:(m + 1) * 128, j * 512:(j + 1) * 512],
                    o_sb[:, j % 2, m, :],
                ).then_inc(s_oS, 16)


        @block.gpsimd
        def _(gpsimd: bass.BassEngine):
            half = MB // 2
            for kt in range(KT):
                if kt < N_SPLIT_X:
                    # upper half; the startup-critical lower half goes via
                    # scalar's HWDGE queue (faster to first byte than SWDGE)
                    gpsimd.dma_start(
                        x_sb[:, kt, half:MB],
                        xT[kt * 128:(kt + 1) * 128, half:MB],
                    ).then_inc(s_x[kt], 16)
                else:
                    gpsimd.dma_start(
                        x_sb[:, kt, :],
                        xT[kt * 128:(kt + 1) * 128, :],
                    ).then_inc(s_x[kt], 16)
            for j in range(NT - 1):
                emit_store_pass(gpsimd, j)
            # last pass: gpsimd handles m3/m5/m7
            j = NT - 1
            for m in (3, MT - 3, MT - 1):
                wsem, wval = ep_wait(j, m)
                gpsimd.wait_ge(wsem, wval)
                gpsimd.dma_start(
                    out[m * 128:(m + 1) * 128, j * 512:(j + 1) * 512],
                    o_sb[:, j % 2, m, :],
                ).then_inc(s_oG, 16)
            # teardown: sync with scalar+vector engine clocks (which carry
            # PE's transitively via their s_mm waits), gate on store
            # completions, then reset DMA state and clear all kernel
            # semaphores in two instructions.
            gpsimd.wait_ge(s_fin, 2)
            gpsimd.wait_ge(s_o[0], o_slot_cum[0])
            if o_slot_cum[1]:
                gpsimd.wait_ge(s_o[1], o_slot_cum[1])
            if not safe_exit:
                gpsimd.dma_reset(sem_range)
                gpsimd.sem_clear(sem_range)
            # store sems (s_oS/s_oG, outside the cleared range) are zeroed by
            # Bacc's defensive full-range reset, which runs after every
            # engine's exit DRAIN — i.e. after both store queues drain.

        @block.scalar
        def _(scalar: bass.BassEngine):
            # startup x halves (lower half of the first N_SPLIT_X k-tiles —
            # the first matmul's critical data, on the HWDGE queue)
            half = MB // 2
            for kt in range(N_SPLIT_X):
                scalar.dma_start(
                    x_sb[:, kt, 0:half],
                    xT[kt * 128:(kt + 1) * 128, 0:half],
                ).then_inc(s_xs[kt], 16)
            for j in range(NT):
                for m in range(0, MT, 2):
                    scalar.wait_ge(s_mm, MT * j + m + 1)
                    if j == 0 and m == 0:
                        scalar.wait_ge(s_bias, 1)
                    if j >= 2:
                        scalar.wait_ge(s_o[j % 2], o_targets[j - 2])
                    scalar.activation(
                        o_sb[:, j % 2, m, :], ps[:, m, :],
                        mybir.ActivationFunctionType.Relu,
                        bias=bias_sb[:], scale=1.0,
                    ).then_inc(s_eps, 1)
                    if j == NT - 1 and m in (2, MT - 2):
                        # inline last-pass store (engine-ordered after ACT)
                        scalar.dma_start(
                            out[m * 128:(m + 1) * 128,
                                j * 512:(j + 1) * 512],
                            o_sb[:, j % 2, m, :],
                        ).then_inc(s_oS, 16)
            scalar.sem_inc(s_fin, 1)

        @block.vector
        def _(vector: bass.BassEngine):
            vector.wait_ge(s_b, 16)
            vector.tensor_scalar(
                bias_sb[:], beta_sb[:], -1.0, -1.0,
                mybir.AluOpType.mult, mybir.AluOpType.subtract,
            ).then_inc(s_bias, 1)
            for j in range(NT):
                for m in range(1, MT, 2):
                    vector.wait_ge(s_mm, MT * j + m + 1)
                    if j >= 2:
                        vector.wait_ge(s_o[j % 2], o_targets[j - 2])
                    vector.tensor_scalar(
                        o_sb[:, j % 2, m, :], ps[:, m, :], bias_sb[:], 0.0,
                        mybir.AluOpType.add, mybir.AluOpType.max,
                    ).then_inc(s_epv, 1)
            vector.sem_inc(s_fin, 1)

        @block.tensor
        def _(tensor: bass.BassEngine):
            # Warm the PE clock (HAM) during the startup DMA window: short
            # dummy matmuls on uninitialized SBUF, results overwritten by the
            # real kt==0 start-group. N=128 keeps each one ~107 ns so they
            # start the HAM busy-window early without delaying the real
            # stream once data lands.
            for _ in range(N_WARMUP_MM):
                tensor.matmul(ps[:, 0, 0:128], x_sb[:, 0, 0:128],
                              w_sb[:, 0, 0:128],
                              start=True, stop=True, skip_group_check=True)
            i = 0
            pending_wu = 0  # w-tile-consumed incs not yet attached (see below)
            for j in range(NT):
                for kt in range(KT):
                    sl = i % W_BUFS
                    tensor.wait_ge(s_w[sl], w_targets[i])
                    if j == 0 and kt < 2:
                        # startup tiles: half-width MMs so compute can begin
                        # on the first 64 KB w chunk; 2nd halves gate on the
                        # chunk's dedicated s_wh sem
                        for m in range(MT):
                            if m == 0:
                                tensor.wait_ge(s_xs[kt], 16)
                            elif m == MT // 2:
                                tensor.wait_ge(s_x[kt], 16)
                            tensor.matmul(
                                ps[:, m, 0:256],
                                x_sb[:, kt, m * 128:(m + 1) * 128],
                                w_sb[:, sl, 0:256],
                                start=(kt == 0),
                                stop=False,
                                skip_group_check=True,
                            )
                            if m == 0:
                                tensor.wait_ge(s_wh[kt], 16)
                            mm = tensor.matmul(
                                ps[:, m, 256:512],
                                x_sb[:, kt, m * 128:(m + 1) * 128],
                                w_sb[:, sl, 256:512],
                                start=False,
                                stop=False,
                                skip_group_check=True,
                            )
                            if m == MT - 1:
                                mm.then_inc(s_wu, 1 + pending_wu)
                                pending_wu = 0
                            elif pending_wu:
                                mm.then_inc(s_wu, pending_wu)
                                pending_wu = 0
                        i += 1
                        continue
                    for m in range(MT):
                        if j == 0:
                            # gate on just the x half covering m's rows:
                            # lower half via scalar (s_xs), upper via gpsimd
                            if m == 0:
                                tensor.wait_ge(
                                    s_xs[kt] if kt < N_SPLIT_X else s_x[kt], 16)
                            elif m == MT // 2 and kt < N_SPLIT_X:
                                tensor.wait_ge(s_x[kt], 16)
                        if kt == 0 and j > 0:
                            wsem, wval = ep_wait(j - 1, m)
                            tensor.wait_ge(wsem, wval)
                        mm = tensor.matmul(
                            ps[:, m, :],
                            x_sb[:, kt, m * 128:(m + 1) * 128],
                            w_sb[:, sl, :],
                            start=(kt == 0),
                            stop=(kt == KT - 1),
                        )
                        # One sem update max per instruction. kt==KT-1 MMs
                        # must carry s_mm (epilogue gating, in (j, m) order),
                        # so the w-consumed inc of a pass's last tile is
                        # deferred to the next pass's first MM — safe because
                        # PE completions are pc-monotone.
                        if kt == KT - 1:
                            mm.then_inc(s_mm, 1)
                        elif m == MT - 1:
                            mm.then_inc(s_wu, 1 + pending_wu)
                            pending_wu = 0
                        elif pending_wu:
                            mm.then_inc(s_wu, pending_wu)
                            pending_wu = 0
                    if kt == KT - 1:
                        pending_wu += 1
                    i += 1

    if safe_exit:
        # CoreSim's race detector requires a full barrier before clearing
        nc.sync.drain()
        nc.all_engine_barrier()
        nc.gpsimd.dma_reset(sem_range)
        nc.gpsimd.sem_clear(sem_range)
        # late range (store sems) left to Bacc's defensive reset; CoreSim
        # never re-executes, and its race detector cannot model DMA-update
        # clocks, so no explicit clear here.
    nc.compile()
    return nc




GRID_B, GRID_O = 4, 2
MB_SHARD, NO_SHARD = 4096 // GRID_B, 4096 // GRID_O

_NC_CACHE = None


def _get_nc():
    global _NC_CACHE
    if _NC_CACHE is None:
        _NC_CACHE = build_raw(IN=4096, MB=MB_SHARD, NO=NO_SHARD, W_BUFS=12)
    return _NC_CACHE


def kernel(x, weights, beta, _trace=False, _results_out=None):
    from concourse.bass_utils import run_bass_kernel_spmd

    x = np.asarray(x, dtype=np.float32)
    weights = np.asarray(weights, dtype=np.float32)
    beta = np.asarray(beta, dtype=np.float32)

    xT = np.ascontiguousarray(x.T.astype(np.float16))        # [IN, BATCH]
    wT = np.ascontiguousarray(weights.T.astype(np.float16))  # [IN, OUT]
    beta_b = np.ascontiguousarray(
        np.broadcast_to(beta.reshape(1, 1), (128, 1)).astype(np.float32)
    )

    in_maps = []
    for c in range(GRID_B * GRID_O):
        bi, oj = divmod(c, GRID_O)
        in_maps.append({
            "xT": np.ascontiguousarray(xT[:, bi * MB_SHARD:(bi + 1) * MB_SHARD]),
            "wT": np.ascontiguousarray(wT[:, oj * NO_SHARD:(oj + 1) * NO_SHARD]),
            "beta": beta_b,
        })

    nc = _get_nc()
    res = run_bass_kernel_spmd(
        nc, in_maps, core_ids=list(range(8)), trace=_trace,
        trace_cores=list(range(8)) if _trace else None,
    )
    if _results_out is not None:
        _results_out.append(res)

    out = np.empty((4096, 4096), dtype=np.float32)
    for c in range(GRID_B * GRID_O):
        bi, oj = divmod(c, GRID_O)
        out[bi * MB_SHARD:(bi + 1) * MB_SHARD,
            oj * NO_SHARD:(oj + 1) * NO_SHARD] = res.results[c]["out"]  # f16 -> f32
    return out

